# revision 23
# baseline (speedup 1.0000x reference)
"""Causal attention kernel for Trainium2, 8-core SPMD.

Problem: B=2 H=16 S=2048 D=64 fp32 causal attention (n_qry == n_tok).
Sharding: B*H = 32 head-slices, 4 per core (data/head parallel); each core
computes full attention for its 4 heads independently.

Per-head scheme (all on one core):
  - Q,K loaded fp32, cast bf16, transposed to [d, seq] layout via DMA-transpose
    (xbar) in [128,128] pair tiles.
  - Scores computed transposed: S^T[k, q] = K^T-chunk.T @ Q^T, k-chunk of 128
    on partitions, q on free dim.  Only q >= 128*chunk computed (causal).
  - exp on ACT engine (scale=1/sqrt(D) folded in), output bf16; the diagonal
    [128,128] staircase corner is zeroed with gpsimd.affine_select.
  - PV: out[q, d] accumulated per q-tile of 128: lhsT = expS^T slice (k x q),
    rhs = V chunk with a ones column appended -> column 64 of the PSUM
    accumulator is the softmax denominator.  Normalize with DVE reciprocal +
    tensor_scalar_mul, DMA out.
"""

import numpy as np

B, H, SEQ, D = 2, 16, 2048, 64
N_CORES = 8
HPC = (B * H) // N_CORES  # heads per core
NT = SEQ // 128  # 16 k-chunks / q-tiles
LSPLIT = 1  # HBM load split (1 = single DMA per tensor)
NOFF = 2  # leading k-chunks per head whose exp runs on DVE instead of ACT


_EXP_FIX = None
# Schraudolph int-round exp + mantissa polynomial correction.
# op1 (stock tensor_scalar): I = int32(round(S*A + B)), A = 2^23*log2e*scale,
#   B = 127*2^23.  y0 = bitcast_f32(I) = 2^(e-127)*(1+theta).
# op2 (EXP_FIX): w = (bits(y0) | bits(1.0)) & 0x3FFFFFFF -> 1+theta in [1,2);
#   out = y0 * ((a2*w + a1)*w + a0) ~= e^(S*scale), max rel err ~5e-3.
EXP_A = float(2.0 ** 23 * 1.4426950408889634 * 0.125)
EXP_B = float(127 * 2 ** 23)
EXP_MASK = float(__import__("numpy").uint32(0x3FFFFFFF).view("float32"))
EXP_C = (1.4744260955454123, -0.7178516282288774, 0.24099145663822977)


def _get_exp_fix():
    global _EXP_FIX
    if _EXP_FIX is not None:
        return _EXP_FIX
    import numpy as np
    from concourse import dve_ops
    from concourse.dve_spec import AluOp, Bin, Spec, Src0, Src1, C0, C1, C2, One
    from concourse.dve_table_gen import dve_ver_for

    _w = Bin(AluOp.BITWISE_AND, Bin(AluOp.BITWISE_OR, Src0, One), C0)

    def _ref(in0, in1, s0, s1, imm2):
        w = ((in0.view(np.int32) | np.float32(1.0).view(np.int32))
             & np.float32(s0).view(np.int32)).view(np.float32)
        return in0 * ((imm2 * w + s1) * w + in1)

    # Newly-appended opcode rows (>16) crash the device firmware, so take
    # over the registry slot (name + opcode row) of a production op this
    # kernel never uses.
    slot_name = "LN_BWD_DX_ANT"
    op = dve_ops.DveOp(
        slot_name,
        Spec(body=((C2 * _w + C1) * _w + Src1) * Src0, reference=_ref),
        subdim=False,
        uops_sha={},
    )
    ver = dve_ver_for("TRN2")
    idx = [i for i, o in enumerate(dve_ops.OPS) if o.name == slot_name]
    dve_ops.OPS[idx[0]] = op
    dve_ops.CUSTOM_DVE_SPECS[slot_name] = op.spec
    dve_ops._COMPILE_CACHE.pop((slot_name, ver), None)
    from concourse import bass_utils as _bu
    _bu._table_cache.clear()
    from concourse.dve_spec import lower
    from concourse.dve_ops import has_src1
    from concourse.dve_uop import DveOpSpec
    tmp = DveOpSpec(name=op.name, opcode=dve_ops.get_dve_sub_opcode(op.name),
                    uops=lower(op.spec, ver=ver), rd1_en=has_src1(op.spec))
    op.uops_sha[ver] = tmp.sha(ver)
    _EXP_FIX = op
    return op



def _build(rep=1):
    from contextlib import ExitStack

    import concourse.bass as bass
    import concourse.mybir as mybir
    import concourse.tile as tile
    from concourse import bacc

    f32 = mybir.dt.float32
    bf16 = mybir.dt.bfloat16

    nc = bacc.Bacc("TRN2", target_bir_lowering=False, debug=False,
                   num_devices=N_CORES)
    Qd = nc.dram_tensor("Q", [HPC, SEQ, D], f32, kind="ExternalInput").ap()
    Kd = nc.dram_tensor("K", [HPC, SEQ, D], f32, kind="ExternalInput").ap()
    Vd = nc.dram_tensor("V", [HPC, SEQ, D], f32, kind="ExternalInput").ap()
    Od = nc.dram_tensor("O", [HPC, SEQ, D], f32, kind="ExternalOutput").ap()

    with tile.TileContext(nc) as tc, ExitStack() as ctx:
        stage = ctx.enter_context(tc.tile_pool(name="stage", bufs=3))
        qkbf = ctx.enter_context(tc.tile_pool(name="qkbf", bufs=2))
        qkt = ctx.enter_context(tc.tile_pool(name="qkt", bufs=2))
        vpool = ctx.enter_context(tc.tile_pool(name="vpool", bufs=2))
        epool = ctx.enter_context(tc.tile_pool(name="exps", bufs=2))
        rdpool = ctx.enter_context(tc.tile_pool(name="rd", bufs=4))
        outp = ctx.enter_context(tc.tile_pool(name="outp", bufs=4))
        ibufp = ctx.enter_context(tc.tile_pool(name="ibufp", bufs=2))
        singles = ctx.enter_context(tc.tile_pool(name="singles", bufs=1))
        spsum = ctx.enter_context(tc.tile_pool(name="spsum", bufs=1, space="PSUM"))
        opsum = ctx.enter_context(tc.tile_pool(name="opsum", bufs=4, space="PSUM"))

        i32 = mybir.dt.int32
        expfix = _get_exp_fix()
        a0t = singles.tile([128, SEQ], f32)
        nc.vector.memset(a0t, EXP_C[0])

        rep_cm = tc.For_i(0, rep, 1) if rep > 1 else None
        if rep_cm is not None:
            rep_cm.__enter__()
        for h in range(HPC):
            # ---- load + cast + transpose Q and K ----
            # Q^T is assembled into one contiguous [64, SEQ] tile so QK^T can
            # run with 512-wide moving operands; K^T stays in pair-tile form
            # (stationary operands are 128 wide anyway).
            tps = []
            for name, src in (("q", Qd), ("k", Kd)):
                fstage = stage.tile([128, NT, D], f32, tag="stage")
                # split the strided load over several DMA instructions so the
                # descriptor processing spreads across HWDGE queues
                for l in range(LSPLIT):
                    cs = NT // LSPLIT
                    nc.sync.dma_start(
                        out=fstage[:, l * cs:(l + 1) * cs, :],
                        in_=src[h].rearrange("(c p) d -> p c d", p=128)
                        [:, l * cs:(l + 1) * cs, :])
                bcast = qkbf.tile([128, NT, D], bf16, tag="qkbf")
                nc.vector.tensor_copy(out=bcast, in_=fstage)
                tp = qkt.tile([128, NT // 2, 128], bf16, tag="qkt")
                for j in range(NT // 2):
                    nc.sync.dma_start(
                        out=tp[:, j, :],
                        in_=bcast[:, 2 * j:2 * j + 2, :].rearrange("p a b -> p (a b)"),
                        transpose=True)
                if name == "q":
                    # assemble contiguous Q^T [64, SEQ] with 2 strided copies
                    qtf = qkt.tile([64, SEQ], bf16, tag="qtf")
                    qv = qtf.rearrange("p (j t f) -> p j t f", t=2, f=128)
                    nc.sync.dma_start(out=qv[:, :, 0, :], in_=tp[0:64, :, :])
                    nc.sync.dma_start(out=qv[:, :, 1, :], in_=tp[64:128, :, :])
                    tps.append(qtf)
                else:
                    # odd chunks land on partitions 64-127; matmul needs both
                    # operands at the same base partition -> copy to base 0
                    todd = qkt.tile([64, NT // 2, 128], bf16, tag="qkt_odd")
                    nc.sync.dma_start(out=todd[:, :, :], in_=tp[64:128, :, :])
                    tps.append((tp, todd))

            def t_chunk(tp_pair, c):
                tp, todd = tp_pair
                if c % 2 == 0:
                    return tp[0:64, c // 2, :]
                return todd[:, c // 2, :]

            QT, KT = tps

            # ---- load + cast V, append ones column ----
            vstage = stage.tile([128, NT, D], f32, tag="stage")
            for l in range(LSPLIT):
                cs = NT // LSPLIT
                nc.sync.dma_start(
                    out=vstage[:, l * cs:(l + 1) * cs, :],
                    in_=Vd[h].rearrange("(c p) d -> p c d", p=128)
                    [:, l * cs:(l + 1) * cs, :])
            Vb = vpool.tile([128, NT, D + 1], bf16, tag="v")
            nc.vector.tensor_copy(out=Vb[:, :, 0:D], in_=vstage)
            nc.vector.memset(Vb[:, :, D:D + 1], 1.0)

            expS = epool.tile([128, NT, SEQ], bf16, tag="e")
            ogs = {}
            for i in range(NT):
                # ---- scores chunk i: S^T[kk, q] for k in [128i, 128i+128),
                #      q in [128i, 2048) ----
                St = spsum.tile([128, SEQ], f32, tag="s")
                lhsT = t_chunk(KT, i)
                # moving operand: contiguous Q^T columns [128i, SEQ) in
                # <=512 pieces aligned to PSUM banks
                q0 = 128 * i
                bounds = [q0] + [b for b in range(512 * (i // 4 + 1), SEQ + 1, 512)]
                for lo, hi in zip(bounds[:-1], bounds[1:]):
                    nc.tensor.matmul(St[:, lo:hi], lhsT, QT[:, lo:hi],
                                     start=True, stop=True)
                if i < NOFF:
                    n_i = SEQ - 128 * i
                    ib = ibufp.tile([128, SEQ], i32, tag="ib")
                    nc.vector.tensor_scalar(
                        out=ib[:, 0:n_i], in0=St[:, 128 * i:SEQ],
                        scalar1=EXP_A, scalar2=EXP_B,
                        op0=mybir.AluOpType.mult, op1=mybir.AluOpType.add)
                    nc.vector._custom_dve(
                        expfix,
                        out=expS[:, i, 128 * i:SEQ],
                        in0=ib[:, 0:n_i].bitcast(f32),
                        in1=a0t[:, 0:n_i],
                        s0=EXP_MASK, s1=EXP_C[1], imm2=EXP_C[2])
                else:
                    nc.scalar.activation(
                        out=expS[:, i, 128 * i:SEQ], in_=St[:, 128 * i:SEQ],
                        func=mybir.ActivationFunctionType.Exp, scale=0.125)
                # zero the strict upper-triangle of the diagonal corner
                # (keep where q - kk >= 0)
                nc.gpsimd.affine_select(
                    out=expS[:, i, 128 * i:128 * (i + 1)],
                    in_=expS[:, i, 128 * i:128 * (i + 1)],
                    compare_op=mybir.AluOpType.is_ge,
                    fill=0.0, base=0, channel_multiplier=-1,
                    pattern=[[1, 128]])

                # ---- PV for q-tile i (all chunks c <= i ready) ----
                g, jj = divmod(i, 4)
                if jj == 0:
                    og_new = opsum.tile([128, 4, D + 1], f32, tag="o")
                    ogs[g] = og_new
                og = ogs[g]
                for c in range(i + 1):
                    nc.tensor.matmul(
                        og[:, jj, :],
                        expS[:, c, 128 * i:128 * (i + 1)],
                        Vb[:, c, :],
                        start=(c == 0), stop=(c == i))

                if jj == 3:
                    rd = rdpool.tile([128, 4], f32, tag="rd")
                    nc.vector.reciprocal(out=rd, in_=og[:, :, D])
                    ot = outp.tile([128, 4, D], f32, tag="ot")
                    for k in range(4):
                        nc.vector.tensor_scalar_mul(ot[:, k, :], og[:, k, 0:D],
                                                    rd[:, k:k + 1])
                    nc.sync.dma_start(
                        out=Od[h].rearrange("(j p) d -> p j d", p=128)
                        [:, 4 * g:4 * g + 4, :],
                        in_=ot)

        if rep_cm is not None:
            rep_cm.__exit__(None, None, None)

    nc.compile()
    return nc


_NC = {}


def _get_nc(rep=1):
    if rep not in _NC:
        _NC[rep] = _build(rep)
    return _NC[rep]


def kernel(Q, K_cache, V_cache):
    from concourse.bass_utils import run_bass_kernel_spmd

    nc = _get_nc()
    Qs = np.ascontiguousarray(np.asarray(Q, dtype=np.float32).reshape(B * H, SEQ, D))
    Ks = np.ascontiguousarray(np.asarray(K_cache, dtype=np.float32).reshape(B * H, SEQ, D))
    Vs = np.ascontiguousarray(np.asarray(V_cache, dtype=np.float32).reshape(B * H, SEQ, D))
    in_maps = []
    for c in range(N_CORES):
        sl = slice(c * HPC, (c + 1) * HPC)
        in_maps.append({"Q": Qs[sl], "K": Ks[sl], "V": Vs[sl]})
    res = run_bass_kernel_spmd(nc, in_maps, list(range(N_CORES)))
    out = np.concatenate([res.results[c]["O"] for c in range(N_CORES)], axis=0)
    return out.reshape(B, H, SEQ, D)


# revision 26
# speedup vs baseline: 740.2268x; 740.2268x over previous
"""Causal attention kernel for Trainium2, 8-core SPMD.

Problem: B=2 H=16 S=2048 D=64 fp32 causal attention (n_qry == n_tok).
Sharding: B*H = 32 head-slices, 4 per core (data/head parallel); each core
computes full attention for its 4 heads independently.

Per-head scheme (all on one core):
  - Q,K loaded fp32, cast bf16, transposed to [d, seq] layout via DMA-transpose
    (xbar) in [128,128] pair tiles.
  - Scores computed transposed: S^T[k, q] = K^T-chunk.T @ Q^T, k-chunk of 128
    on partitions, q on free dim.  Only q >= 128*chunk computed (causal).
  - exp on ACT engine (scale=1/sqrt(D) folded in), output bf16; the diagonal
    [128,128] staircase corner is zeroed with gpsimd.affine_select.
  - PV: out[q, d] accumulated per q-tile of 128: lhsT = expS^T slice (k x q),
    rhs = V chunk with a ones column appended -> column 64 of the PSUM
    accumulator is the softmax denominator.  Normalize with DVE reciprocal +
    tensor_scalar_mul, DMA out.
"""

import numpy as np

B, H, SEQ, D = 2, 16, 2048, 64
N_CORES = 8
HPC = (B * H) // N_CORES  # heads per core
NT = SEQ // 128  # 16 k-chunks / q-tiles
LSPLIT = 1  # HBM load split (1 = single DMA per tensor)
NOFF = 2  # leading k-chunks per head whose exp runs on DVE instead of ACT


_EXP_FIX = None
# Schraudolph int-round exp + mantissa polynomial correction.
# op1 (stock tensor_scalar): I = int32(round(S*A + B)), A = 2^23*log2e*scale,
#   B = 127*2^23.  y0 = bitcast_f32(I) = 2^(e-127)*(1+theta).
# op2 (EXP_FIX): w = (bits(y0) | bits(1.0)) & 0x3FFFFFFF -> 1+theta in [1,2);
#   out = y0 * ((a2*w + a1)*w + a0) ~= e^(S*scale), max rel err ~5e-3.
EXP_A = float(2.0 ** 23 * 1.4426950408889634 * 0.125)
EXP_B = float(127 * 2 ** 23)
EXP_MASK = float(__import__("numpy").uint32(0x3FFFFFFF).view("float32"))
EXP_C = (1.4744260955454123, -0.7178516282288774, 0.24099145663822977)


def _get_exp_fix():
    global _EXP_FIX
    if _EXP_FIX is not None:
        return _EXP_FIX
    import numpy as np
    from concourse import dve_ops
    from concourse.dve_spec import AluOp, Bin, Spec, Src0, Src1, C0, C1, C2, One
    from concourse.dve_table_gen import dve_ver_for

    _w = Bin(AluOp.BITWISE_AND, Bin(AluOp.BITWISE_OR, Src0, One), C0)

    def _ref(in0, in1, s0, s1, imm2):
        w = ((in0.view(np.int32) | np.float32(1.0).view(np.int32))
             & np.float32(s0).view(np.int32)).view(np.float32)
        return in0 * ((imm2 * w + s1) * w + in1)

    # Newly-appended opcode rows (>16) crash the device firmware, so take
    # over the registry slot (name + opcode row) of a production op this
    # kernel never uses.
    slot_name = "LN_BWD_DX_ANT"
    op = dve_ops.DveOp(
        slot_name,
        Spec(body=((C2 * _w + C1) * _w + Src1) * Src0, reference=_ref),
        subdim=False,
        uops_sha={},
    )
    ver = dve_ver_for("TRN2")
    idx = [i for i, o in enumerate(dve_ops.OPS) if o.name == slot_name]
    dve_ops.OPS[idx[0]] = op
    dve_ops.CUSTOM_DVE_SPECS[slot_name] = op.spec
    dve_ops._COMPILE_CACHE.pop((slot_name, ver), None)
    from concourse import bass_utils as _bu
    _bu._table_cache.clear()
    from concourse.dve_spec import lower
    from concourse.dve_ops import has_src1
    from concourse.dve_uop import DveOpSpec
    tmp = DveOpSpec(name=op.name, opcode=dve_ops.get_dve_sub_opcode(op.name),
                    uops=lower(op.spec, ver=ver), rd1_en=has_src1(op.spec))
    op.uops_sha[ver] = tmp.sha(ver)
    _EXP_FIX = op
    return op



def _build(rep=1):
    from contextlib import ExitStack

    import concourse.bass as bass
    import concourse.mybir as mybir
    import concourse.tile as tile
    from concourse import bacc

    f32 = mybir.dt.float32
    bf16 = mybir.dt.bfloat16

    nc = bacc.Bacc("TRN2", target_bir_lowering=False, debug=False,
                   num_devices=N_CORES)
    Qd = nc.dram_tensor("Q", [HPC, SEQ, D], f32, kind="ExternalInput").ap()
    Kd = nc.dram_tensor("K", [HPC, SEQ, D], f32, kind="ExternalInput").ap()
    Vd = nc.dram_tensor("V", [HPC, SEQ, D], f32, kind="ExternalInput").ap()
    Od = nc.dram_tensor("O", [HPC, SEQ, D], f32, kind="ExternalOutput").ap()

    with tile.TileContext(nc) as tc, ExitStack() as ctx:
        stage = ctx.enter_context(tc.tile_pool(name="stage", bufs=3))
        qkbf = ctx.enter_context(tc.tile_pool(name="qkbf", bufs=2))
        qkt = ctx.enter_context(tc.tile_pool(name="qkt", bufs=2))
        vpool = ctx.enter_context(tc.tile_pool(name="vpool", bufs=2))
        epool = ctx.enter_context(tc.tile_pool(name="exps", bufs=2))
        rdpool = ctx.enter_context(tc.tile_pool(name="rd", bufs=4))
        outp = ctx.enter_context(tc.tile_pool(name="outp", bufs=4))
        ibufp = ctx.enter_context(tc.tile_pool(name="ibufp", bufs=2))
        singles = ctx.enter_context(tc.tile_pool(name="singles", bufs=1))
        spsum = ctx.enter_context(tc.tile_pool(name="spsum", bufs=3, space="PSUM"))
        opsum = ctx.enter_context(tc.tile_pool(name="opsum", bufs=2, space="PSUM"))

        i32 = mybir.dt.int32
        expfix = _get_exp_fix()
        a0t = singles.tile([128, SEQ], f32)
        nc.vector.memset(a0t, EXP_C[0])

        rep_cm = tc.For_i(0, rep, 1) if rep > 1 else None
        if rep_cm is not None:
            rep_cm.__enter__()
        for h in range(HPC):
            # ---- load + cast + transpose Q and K ----
            # Q^T is assembled into one contiguous [64, SEQ] tile so QK^T can
            # run with 512-wide moving operands; K^T stays in pair-tile form
            # (stationary operands are 128 wide anyway).
            tps = []
            for name, src in (("q", Qd), ("k", Kd)):
                fstage = stage.tile([128, NT, D], f32, tag="stage")
                # split the strided load over several DMA instructions so the
                # descriptor processing spreads across HWDGE queues
                for l in range(LSPLIT):
                    cs = NT // LSPLIT
                    nc.sync.dma_start(
                        out=fstage[:, l * cs:(l + 1) * cs, :],
                        in_=src[h].rearrange("(c p) d -> p c d", p=128)
                        [:, l * cs:(l + 1) * cs, :])
                bcast = qkbf.tile([128, NT, D], bf16, tag="qkbf")
                nc.vector.tensor_copy(out=bcast, in_=fstage)
                tp = qkt.tile([128, NT // 2, 128], bf16, tag="qkt")
                for j in range(NT // 2):
                    nc.sync.dma_start(
                        out=tp[:, j, :],
                        in_=bcast[:, 2 * j:2 * j + 2, :].rearrange("p a b -> p (a b)"),
                        transpose=True)
                if name == "q":
                    # assemble contiguous Q^T [64, SEQ] with 2 strided copies
                    qtf = qkt.tile([64, SEQ], bf16, tag="qtf")
                    qv = qtf.rearrange("p (j t f) -> p j t f", t=2, f=128)
                    nc.sync.dma_start(out=qv[:, :, 0, :], in_=tp[0:64, :, :])
                    nc.sync.dma_start(out=qv[:, :, 1, :], in_=tp[64:128, :, :])
                    tps.append(qtf)
                else:
                    # odd chunks land on partitions 64-127; matmul needs both
                    # operands at the same base partition -> copy to base 0
                    todd = qkt.tile([64, NT // 2, 128], bf16, tag="qkt_odd")
                    nc.sync.dma_start(out=todd[:, :, :], in_=tp[64:128, :, :])
                    tps.append((tp, todd))

            def t_chunk(tp_pair, c):
                tp, todd = tp_pair
                if c % 2 == 0:
                    return tp[0:64, c // 2, :]
                return todd[:, c // 2, :]

            QT, KT = tps

            # ---- load + cast V, append ones column ----
            vstage = stage.tile([128, NT, D], f32, tag="stage")
            for l in range(LSPLIT):
                cs = NT // LSPLIT
                nc.sync.dma_start(
                    out=vstage[:, l * cs:(l + 1) * cs, :],
                    in_=Vd[h].rearrange("(c p) d -> p c d", p=128)
                    [:, l * cs:(l + 1) * cs, :])
            Vb = vpool.tile([128, NT, D + 1], bf16, tag="v")
            nc.vector.tensor_copy(out=Vb[:, :, 0:D], in_=vstage)
            nc.vector.memset(Vb[:, :, D:D + 1], 1.0)

            expS = epool.tile([128, NT, SEQ], bf16, tag="e")
            ogs = {}

            def emit_pv(i, _ogs=None):
                g, jj = divmod(i, 4)
                if jj == 0:
                    og_new = opsum.tile([128, 4, D + 1], f32, tag="o")
                    ogs[g] = og_new
                og = ogs[g]
                for c in range(i + 1):
                    nc.tensor.matmul(
                        og[:, jj, :],
                        expS[:, c, 128 * i:128 * (i + 1)],
                        Vb[:, c, :],
                        start=(c == 0), stop=(c == i))
                if jj == 3:
                    rd = rdpool.tile([128, 4], f32, tag="rd")
                    nc.vector.reciprocal(out=rd, in_=og[:, :, D])
                    ot = outp.tile([128, 4, D], f32, tag="ot")
                    rdb = bass.AP(tensor=rd.tensor, offset=rd.offset,
                                  ap=[rd.ap[0], rd.ap[1][:], [0, D]])
                    nc.vector.tensor_tensor(
                        out=ot, in0=og[:, :, 0:D], in1=rdb,
                        op=mybir.AluOpType.mult)
                    nc.sync.dma_start(
                        out=Od[h].rearrange("(j p) d -> p j d", p=128)
                        [:, 4 * g:4 * g + 4, :],
                        in_=ot)

            for i in range(NT):
                # ---- scores chunk i: S^T[kk, q] for k in [128i, 128i+128),
                #      q in [128i, 2048) ----
                # scores in piece-wise PSUM tiles (<=1024 wide) so the next
                # chunk's matmuls can start while this chunk's exp still reads
                lhsT = t_chunk(KT, i)
                q0 = 128 * i
                pieces = ([(q0, 1024), (1024, SEQ)] if q0 < 1024
                          else [(q0, SEQ)])
                for plo, phi in pieces:
                    St = spsum.tile([128, 1024], f32, tag="s")
                    n_p = phi - plo
                    rb = list(range(0, n_p, 512)) + [n_p]
                    for lo, hi in zip(rb[:-1], rb[1:]):
                        nc.tensor.matmul(St[:, lo:hi], lhsT,
                                         QT[:, plo + lo:plo + hi],
                                         start=True, stop=True)
                    if i < NOFF:
                        ib = ibufp.tile([128, 1024], i32, tag="ib")
                        nc.vector.tensor_scalar(
                            out=ib[:, 0:n_p], in0=St[:, 0:n_p],
                            scalar1=EXP_A, scalar2=EXP_B,
                            op0=mybir.AluOpType.mult, op1=mybir.AluOpType.add)
                        nc.vector._custom_dve(
                            expfix,
                            out=expS[:, i, plo:phi],
                            in0=ib[:, 0:n_p].bitcast(f32),
                            in1=a0t[:, 0:n_p],
                            s0=EXP_MASK, s1=EXP_C[1], imm2=EXP_C[2])
                    else:
                        nc.scalar.activation(
                            out=expS[:, i, plo:phi], in_=St[:, 0:n_p],
                            func=mybir.ActivationFunctionType.Exp, scale=0.125)
                # zero the strict upper-triangle of the diagonal corner
                # (keep where q - kk >= 0)
                nc.gpsimd.affine_select(
                    out=expS[:, i, 128 * i:128 * (i + 1)],
                    in_=expS[:, i, 128 * i:128 * (i + 1)],
                    compare_op=mybir.AluOpType.is_ge,
                    fill=0.0, base=0, channel_multiplier=-1,
                    pattern=[[1, 128]])

                # ---- PV one chunk behind, so the PE never stalls on
                # exp(i) while QK(i+1) work is available ----
                if i > 0:
                    emit_pv(i - 1)
            emit_pv(NT - 1)

        if rep_cm is not None:
            rep_cm.__exit__(None, None, None)

    nc.compile()
    return nc


_NC = {}


def _get_nc(rep=1):
    if rep not in _NC:
        _NC[rep] = _build(rep)
    return _NC[rep]


def kernel(Q, K_cache, V_cache):
    from concourse.bass_utils import run_bass_kernel_spmd

    nc = _get_nc()
    Qs = np.ascontiguousarray(np.asarray(Q, dtype=np.float32).reshape(B * H, SEQ, D))
    Ks = np.ascontiguousarray(np.asarray(K_cache, dtype=np.float32).reshape(B * H, SEQ, D))
    Vs = np.ascontiguousarray(np.asarray(V_cache, dtype=np.float32).reshape(B * H, SEQ, D))
    in_maps = []
    for c in range(N_CORES):
        sl = slice(c * HPC, (c + 1) * HPC)
        in_maps.append({"Q": Qs[sl], "K": Ks[sl], "V": Vs[sl]})
    res = run_bass_kernel_spmd(nc, in_maps, list(range(N_CORES)))
    out = np.concatenate([res.results[c]["O"] for c in range(N_CORES)], axis=0)
    return out.reshape(B, H, SEQ, D)
